# revision 47
# baseline (speedup 1.0000x reference)
"""Gated linear attention on 8 TRN2 NeuronCores.

Sharding: data-parallel over tokens. Core c handles tokens
[c*2048, (c+1)*2048) of the flattened (B*N, C) = (16384, 1024) sequence.
The linear-attention kv state (and k-sum) needs a reduction over each
batch's full sequence, so cores {2b, 2b+1} all-reduce a small (128, 520)
bf16 buffer; everything else is local.

Precision: the q, k and gate projections run as fp8e4m3 DoubleRow matmuls
(both operands fp8, K=256 per matmul) -- quantization error on those paths
is compressed by the sigmoid/elu nonlinearities and the q*kv / q*ksum
ratio, measured end-to-end rel-err ~6e-3. The v projection and the output
projection stay bf16 (their error passes straight through).

Engine/queue layout:
  sync   queue: wg8/xt8 chunked loads, wq8, gate transposes
  vector queue: wk8, wv, streamed bf16 x tiles (v path), wp
  gpsimd queue: v_aug/bc copies, AR bounce DMAs, bd scatter DMAs
  scalar queue: output tile DMAs (contiguous y4 layout, host reassembles)
A burst of dummy matmuls at t=0 warms the PE HAM clock gate before the
first real matmul.
"""

import numpy as np
import ml_dtypes

import concourse.bass as bass
import concourse.bacc as bacc
import concourse.tile as tile
import concourse.mybir as mybir
from concourse.bass_utils import run_bass_kernel_spmd

F32 = mybir.dt.float32
BF16 = mybir.dt.bfloat16
F8 = mybir.dt.float8e4
AF = mybir.ActivationFunctionType
ALU = mybir.AluOpType
DR = mybir.MatmulPerfMode.DoubleRow

B, N, C = 4, 4096, 1024
H, D = 16, 64
NCORES = 8
T = B * N // NCORES          # 2048 tokens per core
KC = C // 128                # 8 contraction chunks
KD = KC // 2                 # 4 DoubleRow chunks (K=256 each)
TB = 512                     # token tile (free dim)
NT = T // TB                 # 4 token tiles
NS = T // 128                # 16 token subchunks (partition-dim tiles)
WS = 64.0                    # fp8 weight scale
WSI = 1.0 / WS

REPLICA_GROUPS = [[0, 1], [2, 3], [4, 5], [6, 7]]


def build_nc():
    nc = bacc.Bacc(
        "TRN2", target_bir_lowering=False, debug=False, num_devices=NCORES
    )
    xt8 = nc.dram_tensor("xt8", [NT * 128, KC * TB], F8, kind="ExternalInput")
    xbt = nc.dram_tensor("xbt", [NT * 128, KC * TB], BF16, kind="ExternalInput")
    wq8 = nc.dram_tensor("wq8", [128, KC * C], F8, kind="ExternalInput")
    wk8 = nc.dram_tensor("wk8", [128, KC * C], F8, kind="ExternalInput")
    wg8 = nc.dram_tensor("wg8", [128, KC * C], F8, kind="ExternalInput")
    wv = nc.dram_tensor("wv", [128, KC * C], BF16, kind="ExternalInput")
    wp = nc.dram_tensor("wp", [128, KC * C], BF16, kind="ExternalInput")
    bg2 = nc.dram_tensor("bg2", [128, KC], F32, kind="ExternalInput")
    bp2 = nc.dram_tensor("bp2", [128, KC], F32, kind="ExternalInput")
    e_all = nc.dram_tensor("e_all", [H, C], BF16, kind="ExternalInput")
    y5 = nc.dram_tensor("y5", [128, NT * KC * TB], BF16, kind="ExternalOutput")

    with tile.TileContext(nc) as tc:
        build_body(nc, tc, xt8, xbt, wq8, wk8, wg8, wv, wp, bg2, bp2, e_all, y5)

    nc.compile()
    return nc


def build_body(nc, tc, xt8d, xbtd, wq8d, wk8d, wg8d, wvd, wpd, bg2, bp2, e_all, y5):
    from contextlib import ExitStack

    with ExitStack() as st:
        constp = st.enter_context(tc.tile_pool(name="constp", bufs=1))
        xp = st.enter_context(tc.tile_pool(name="xp", bufs=1))
        w8p = st.enter_context(tc.tile_pool(name="w8p", bufs=1))
        wbfp = st.enter_context(tc.tile_pool(name="wbfp", bufs=1))
        gatesp = st.enter_context(tc.tile_pool(name="gatesp", bufs=1))
        qp = st.enter_context(tc.tile_pool(name="qp", bufs=1))
        xbp = st.enter_context(tc.tile_pool(name="xbp", bufs=2))
        workp = st.enter_context(tc.tile_pool(name="workp", bufs=2))
        elup = st.enter_context(tc.tile_pool(name="elup", bufs=2))
        attp = st.enter_context(tc.tile_pool(name="attp", bufs=2))
        mmps = st.enter_context(tc.tile_pool(name="mmps", bufs=6, space="PSUM"))
        einps = st.enter_context(tc.tile_pool(name="einps", bufs=2, space="PSUM"))
        dramp = st.enter_context(tc.tile_pool(name="dramp", bufs=1, space="DRAM"))

        # ------------------------------------------------ PE warmup
        # ~45 dummy matmuls keep the PE busy from t~0 so the HAM clock gate
        # reaches K=8/8 before the first real matmul retires.
        warm_in = constp.tile([128, 128], BF16, name="warm_in")
        nc.vector.memset(warm_in[:], 0.0)
        warm_ps = mmps.tile([128, 128], F32, name="warm_ps", tag="mm")
        for _ in range(45):
            nc.tensor.matmul(
                warm_ps[:], lhsT=warm_in[:], rhs=warm_in[:], start=True, stop=True
            )

        # ------------------------------------------------ constants / zeroed state
        # tiny loads on the gpsimd queue; sync stays clear for xt8
        bg_sb = constp.tile([128, KC], F32, name="bg_sb")
        nc.gpsimd.dma_start(bg_sb[:], bg2[:])
        bp_sb = constp.tile([128, KC], F32, name="bp_sb")
        nc.gpsimd.dma_start(bp_sb[:], bp2[:])
        # v-path inputs on the (otherwise idle) gpsimd queue
        wv_sb = wbfp.tile([128, KC, C], BF16, name="wv_sb", tag="wbf")
        nc.gpsimd.dma_start(wv_sb[:], wvd.rearrange("p (c n) -> p c n", c=KC))
        xbtv = xbtd.rearrange("(g p) n -> p g n", p=128)
        xb_tiles = {}
        for g in range(2):
            xb_g = xbp.tile([128, KC, TB], BF16, name="xb_g", tag="xb")
            nc.gpsimd.dma_start(
                xb_g[:], xbtv[:, g, :].rearrange("p (c t) -> p c t", c=KC)
            )
            xb_tiles[g] = xb_g
        e_sb = constp.tile([H, C], BF16, name="e_sb")
        nc.gpsimd.dma_start(e_sb[:], e_all[:])
        bd_all = constp.tile([128, KC, 128], BF16, name="bd_all")
        nc.vector.memset(bd_all[:], 0.0)
        blk_all = constp.tile([128, KC, H], BF16, name="blk_all")
        nc.vector.memset(blk_all[:], 0.0)
        bd_p = constp.tile([128, KC, 128], BF16, name="bd_p")
        nc.vector.memset(bd_p[:], 0.0)
        blk_p = constp.tile([128, KC, H], BF16, name="blk_p")
        nc.vector.memset(blk_p[:], 0.0)

        # gates-path fp8 inputs: xt8 token-sliced on the sync queue (critical;
        # the first gates matmuls need only the first 0.5MB slice), weights
        # chunked on the scalar queue
        wg8_sb = w8p.tile([128, KC, C], F8, name="wg8_sb", tag="w8")
        # token-block-major so each slice load writes SBUF contiguously
        xt8_sb = xp.tile([128, NT, KC, TB], F8, name="xt8_sb")
        wg8v = wg8d.rearrange("p (c n) -> p c n", c=KC)
        xt8v = xt8d.rearrange("(g p) n -> p g n", p=128)
        # wg8 in consumption-order 64KB pieces so the first gates round only
        # waits on ~0.25MB of weights
        for mp in range(KC // 2):
            for kk in range(KD):
                nc.scalar.dma_start(
                    wg8_sb[:, 2 * kk : 2 * kk + 2, mp * 256 : (mp + 1) * 256],
                    wg8v[:, 2 * kk : 2 * kk + 2, mp * 256 : (mp + 1) * 256],
                )
        for n in range(NT):
            nc.sync.dma_start(
                xt8_sb[:, n, :, :],
                xt8v[:, n, :].rearrange("p (c t) -> p c t", c=KC),
            )
        # kv-path fp8 weights next on the scalar queue (needed from ~40us)
        wk8_sb = w8p.tile([128, KC, C], F8, name="wk8_sb")
        nc.scalar.dma_start(wk8_sb[:], wk8d.rearrange("p (c n) -> p c n", c=KC))

        # ------------------------------------------------ phase 1: gates
        # gates[g, tok] = sigmoid((x @ Wg)/WS + bg)^T, feature-major.
        # Token-tile outer loop so compute starts as soon as the first xt8
        # slice lands; transposes stream per (n, m) on the then-idle sync
        # queue: gT_full[p, m, s, c] = gates[m*128+c, s*128+p]
        gates_sb = gatesp.tile([128, KC, T], BF16, name="gates_sb", tag="gbuf")
        gT_full = qp.tile([128, KC, NS, 128], BF16, name="gT_full", tag="qp")
        for n in range(NT):
            for mp in range(KC // 2):
                gps = [
                    mmps.tile([128, TB], F32, name=f"gps{i}", tag="mm")
                    for i in range(2)
                ]
                for kk in range(KD):
                    rhs = xt8_sb[:, n, 2 * kk : 2 * kk + 2, :]
                    for i in range(2):
                        m = 2 * mp + i
                        nc.tensor.matmul(
                            gps[i][:],
                            lhsT=wg8_sb[:, 2 * kk : 2 * kk + 2, m * 128 : (m + 1) * 128],
                            rhs=rhs,
                            start=(kk == 0),
                            stop=(kk == KD - 1),
                            perf_mode=DR,
                        )
                for i in range(2):
                    m = 2 * mp + i
                    nc.scalar.activation(
                        gates_sb[:, m, n * TB : (n + 1) * TB],
                        gps[i][:],
                        AF.Sigmoid,
                        bias=bg_sb[:, m : m + 1],
                        scale=WSI,
                    )
                    nc.sync.dma_start(
                        gT_full[:, m, 4 * n : 4 * n + 4, :],
                        gates_sb[:, m, n * TB : (n + 1) * TB],
                        transpose=True,
                    )
            # stagger the q weights into the scalar queue behind the
            # gates-critical transfers
            if n == 0:
                wq8_sb = w8p.tile([128, KC, C], F8, name="wq8_sb", tag="w8")
                nc.scalar.dma_start(
                    wq8_sb[:], wq8d.rearrange("p (c n) -> p c n", c=KC)
                )

        # ------------------------------------------------ phase 1: k/v + kv state
        # kv_acc block p = cols [130p, 130p+130):
        #   rows 0:64,  cols +0:65   = kv_aug head 2p   (col 64 = k_sum)
        #   rows 64:128, cols +65:130 = kv_aug head 2p+1 (col 129 = k_sum)
        kv_acc = constp.tile([128, KC * 130], F32, name="kv_acc")

        def emit_kproj(s):
            # k projection (fp8 DoubleRow), token-major: out [tok, 1024],
            # then k = elu(k_raw * g) + 1 = min(exp(kg),1) + max(kg,0)
            kps = [
                mmps.tile([128, TB], F32, name=f"kps{nf}", tag="mm")
                for nf in range(2)
            ]
            for kk in range(KD):
                lhsT = xt8_sb[:, s // 4, 2 * kk : 2 * kk + 2,
                              (s % 4) * 128 : (s % 4 + 1) * 128]
                for nf in range(2):
                    nc.tensor.matmul(
                        kps[nf][:],
                        lhsT=lhsT,
                        rhs=wk8_sb[:, 2 * kk : 2 * kk + 2, nf * TB : (nf + 1) * TB],
                        start=(kk == 0),
                        stop=(kk == KD - 1),
                        perf_mode=DR,
                    )
            k_bf = workp.tile([128, C], BF16, name="k_bf", tag="k_bf", bufs=8)
            for nf in range(2):
                kg = elup.tile([128, TB], BF16, name="kg", tag="kg")
                nc.vector.scalar_tensor_tensor(
                    kg.rearrange("p (m c) -> p m c", c=128),
                    in0=kps[nf].rearrange("p (m c) -> p m c", c=128),
                    scalar=WSI,
                    in1=gT_full[:, 4 * nf : 4 * nf + 4, s, :],
                    op0=ALU.mult,
                    op1=ALU.mult,
                )
                relu = elup.tile([128, TB], BF16, name="relu", tag="relu")
                nc.vector.tensor_scalar_max(relu[:], kg[:], 0.0)
                ex = elup.tile([128, TB], BF16, name="ex", tag="ex")
                nc.scalar.activation(ex[:], kg[:], AF.Exp)
                nc.vector.scalar_tensor_tensor(
                    k_bf[:, nf * TB : (nf + 1) * TB],
                    in0=ex[:],
                    scalar=1.0,
                    in1=relu[:],
                    op0=ALU.min,
                    op1=ALU.add,
                )
            return k_bf

        # k-projections run two groups ahead of the v-path: they depend only
        # on xt8/wk8/gT, so they keep the PE fed while the streamed bf16 x
        # tiles (v path) are still in flight.
        kbf_all = {}
        for s in range(8):
            kbf_all[s] = emit_kproj(s)

        for g in range(NT):
            xb_g = xb_tiles[g]
            if g + 2 < NT:
                xb_n = xbp.tile([128, KC, TB], BF16, name="xb_g", tag="xb")
                nc.gpsimd.dma_start(
                    xb_n[:], xbtv[:, g + 2, :].rearrange("p (c t) -> p c t", c=KC)
                )
                xb_tiles[g + 2] = xb_n
            vaugs = []
            for si in range(4):
                # v projection (bf16), token-major: out [tok, 1024]
                vps = [
                    mmps.tile([128, TB], F32, name=f"vps{nf}", tag="mm")
                    for nf in range(2)
                ]
                for k in range(KC):
                    lhsT = xb_g[:, k, si * 128 : (si + 1) * 128]
                    for nf in range(2):
                        nc.tensor.matmul(
                            vps[nf][:],
                            lhsT=lhsT,
                            rhs=wv_sb[:, k, nf * TB : (nf + 1) * TB],
                            start=(k == 0),
                            stop=(k == KC - 1),
                        )
                # v, augmented with ones column per head (yields k_sum)
                v_aug = workp.tile(
                    [128, H * 65], BF16, name="v_aug", tag="v_aug", bufs=4
                )
                v3 = v_aug.rearrange("p (h e) -> p h e", e=65)
                nc.gpsimd.memset(v3[:, :, 64:65], 1.0)
                for nf in range(2):
                    h0 = nf * 8
                    nc.scalar.copy(
                        v3[:, h0 : h0 + 8, 0:64],
                        vps[nf].rearrange("p (h e) -> p h e", e=64),
                    )
                vaugs.append(v_aug)
            # kv einsum for this 512-token group, head pairs packed [128, 130]
            for p in range(KC):
                eps = einps.tile([128, 130], F32, name="eps", tag="ein")
                for si in range(4):
                    nc.tensor.matmul(
                        eps[:],
                        lhsT=kbf_all[g * 4 + si][:, 128 * p : 128 * (p + 1)],
                        rhs=vaugs[si][:, 130 * p : 130 * (p + 1)],
                        start=(si == 0),
                        stop=(si == 3),
                    )
                if g == 0:
                    nc.vector.tensor_copy(
                        kv_acc[:, 130 * p : 130 * (p + 1)], eps[:]
                    )
                else:
                    nc.vector.tensor_add(
                        kv_acc[:, 130 * p : 130 * (p + 1)],
                        kv_acc[:, 130 * p : 130 * (p + 1)],
                        eps[:],
                    )
            # next-next group's k projections follow this group's einsum
            for s in range((g + 2) * 4, min((g + 3) * 4, NS)):
                kbf_all[s] = emit_kproj(s)

        # proj weights next on the scalar queue (reuses the wv slot)
        wp_sb = wbfp.tile([128, KC, C], BF16, name="wp_sb", tag="wbf")
        nc.scalar.dma_start(wp_sb[:], wpd.rearrange("p (c n) -> p c n", c=KC))

        # ------------------------------------------------ kv all-reduce (pairs)
        # compact bf16 payload [128, 8*65]: head 2p at [0:64, 65p:65p+65],
        # head 2p+1 at [64:128, 65p:65p+65]. DVE casts (fast), gpsimd owns
        # the bounce DMAs so nothing queues behind the big transfer queues.
        kv_cat = constp.tile([128, KC * 65], BF16, name="kv_cat")
        nc.vector.tensor_copy(
            kv_cat[0:64, :].rearrange("p (j e) -> p j e", e=65),
            kv_acc[0:64, :].rearrange("p (j q) -> p j q", q=130)[:, :, 0:65],
        )
        nc.vector.tensor_copy(
            kv_cat[64:128, :].rearrange("p (j e) -> p j e", e=65),
            kv_acc[64:128, :].rearrange("p (j q) -> p j q", q=130)[:, :, 65:130],
        )
        bounce_in = dramp.tile([128, KC * 65], BF16, name="bounce_in")
        bounce_out = dramp.tile([128, KC * 65], BF16, name="bounce_out")
        nc.gpsimd.dma_start(bounce_in[:], kv_cat[:])
        nc.gpsimd.collective_compute(
            "AllReduce",
            ALU.add,
            replica_groups=REPLICA_GROUPS,
            ins=[bounce_in.opt()],
            outs=[bounce_out.opt()],
        )
        # local block-diagonal kv / k_sum tiles, built from kv_cat before the
        # all-reduce lands -- the local halves of attention and normalizer run
        # during the transport; the peer halves accumulate after arrival
        # (bd_peer = bd_sum - bd_local by linearity).
        kv_cat3 = kv_cat.rearrange("p (j e) -> p j e", e=65)
        nc.vector.tensor_copy(bd_all[0:64, :, 0:64], kv_cat3[0:64, :, 0:64])
        nc.vector.tensor_copy(bd_all[64:128, :, 64:128], kv_cat3[64:128, :, 0:64])
        for j in range(KC):
            nc.vector.tensor_copy(
                blk_all[0:64, j, 2 * j : 2 * j + 1], kv_cat3[0:64, j, 64:65]
            )
            nc.vector.tensor_copy(
                blk_all[64:128, j, 2 * j + 1 : 2 * j + 2], kv_cat3[64:128, j, 64:65]
            )

        # ------------------------------------------------ phase 1.5: q (overlaps AR)
        q_sb = qp.tile([128, KC, T], BF16, name="q_sb", tag="qp")
        for m in range(KC):
            qps = [
                mmps.tile([128, TB], F32, name=f"qps{n}", tag="mm")
                for n in range(NT)
            ]
            for kk in range(KD):
                lhsT = wq8_sb[:, 2 * kk : 2 * kk + 2, m * 128 : (m + 1) * 128]
                for n in range(NT):
                    nc.tensor.matmul(
                        qps[n][:],
                        lhsT=lhsT,
                        rhs=xt8_sb[:, n, 2 * kk : 2 * kk + 2, :],
                        start=(kk == 0),
                        stop=(kk == KD - 1),
                        perf_mode=DR,
                    )
            for n in range(NT):
                qg = elup.tile([128, TB], BF16, name="qg", tag="kg")
                nc.vector.scalar_tensor_tensor(
                    qg[:],
                    in0=qps[n][:],
                    scalar=WSI,
                    in1=gates_sb[:, m, n * TB : (n + 1) * TB],
                    op0=ALU.mult,
                    op1=ALU.mult,
                )
                relu = elup.tile([128, TB], BF16, name="relu2", tag="relu")
                nc.vector.tensor_scalar_max(relu[:], qg[:], 0.0)
                ex = elup.tile([128, TB], BF16, name="ex2", tag="ex")
                nc.scalar.activation(ex[:], qg[:], AF.Exp)
                nc.vector.scalar_tensor_tensor(
                    q_sb[:, m, n * TB : (n + 1) * TB],
                    in0=ex[:],
                    scalar=1.0,
                    in1=relu[:],
                    op0=ALU.min,
                    op1=ALU.add,
                )

        # ------------------------------------------------ local attention half
        # attn_loc[n, j] = q_j @ bd_local and norm_loc = q . ksum_local need
        # no peer data: they execute right after the q phase, covering the
        # all-reduce transport and pair skew with real matmuls (also keeping
        # the PE clock gate warm across the phase boundary).
        attn_loc = gatesp.tile([128, NT, KC, TB], BF16, name="attn_loc", tag="gbuf")
        norm_loc = constp.tile([H, NT * TB], F32, name="norm_loc")
        for j in range(KC):
            # all four token chunks per weight so LDWEIGHTS isn't thrashed
            pls = []
            for n in range(NT):
                pl = mmps.tile([128, TB], F32, name="pl", tag="mm")
                nc.tensor.matmul(
                    pl[:],
                    lhsT=bd_all[:, j, :],
                    rhs=q_sb[:, j, n * TB : (n + 1) * TB],
                    start=True,
                    stop=True,
                )
                pls.append(pl)
            for n in range(NT):
                nc.vector.tensor_copy(attn_loc[:, n, j, :], pls[n][:])
            for n in range(NT):
                nl = mmps.tile([H, TB], F32, name="nl", tag="mm")
                nc.tensor.matmul(
                    nl[:],
                    lhsT=blk_all[:, j, :],
                    rhs=q_sb[:, j, n * TB : (n + 1) * TB],
                    start=True,
                    stop=True,
                )
                if j == 0:
                    nc.vector.tensor_copy(
                        norm_loc[:, n * TB : (n + 1) * TB], nl[:]
                    )
                else:
                    nc.vector.tensor_add(
                        norm_loc[:, n * TB : (n + 1) * TB],
                        norm_loc[:, n * TB : (n + 1) * TB],
                        nl[:],
                    )

        # ------------------------------------------------ phase 2: peer + proj
        # pull the reduced state, recover the peer contribution by
        # subtraction, and build the peer block-diagonal tiles
        kv_sum = constp.tile([128, KC * 65], BF16, name="kv_sum")
        nc.gpsimd.dma_start(kv_sum[:], bounce_out[:])
        kv_peer = constp.tile([128, KC * 65], BF16, name="kv_peer")
        nc.vector.tensor_sub(kv_peer[:], kv_sum[:], kv_cat[:])
        kv_p3 = kv_peer.rearrange("p (j e) -> p j e", e=65)
        nc.vector.tensor_copy(bd_p[0:64, :, 0:64], kv_p3[0:64, :, 0:64])
        nc.vector.tensor_copy(bd_p[64:128, :, 64:128], kv_p3[64:128, :, 0:64])
        for j in range(KC):
            nc.vector.tensor_copy(
                blk_p[0:64, j, 2 * j : 2 * j + 1], kv_p3[0:64, j, 64:65]
            )
            nc.vector.tensor_copy(
                blk_p[64:128, j, 2 * j + 1 : 2 * j + 2], kv_p3[64:128, j, 64:65]
            )

        # normalizer: peer half accumulated in PSUM, finalized with the local
        # half, then reciprocals for all token chunks up front
        rbs = []
        for n in range(NT):
            nps = mmps.tile([H, TB], F32, name="nps", tag="mm")
            for j in range(KC):
                nc.tensor.matmul(
                    nps[:],
                    lhsT=blk_p[:, j, :],
                    rhs=q_sb[:, j, n * TB : (n + 1) * TB],
                    start=(j == 0),
                    stop=(j == KC - 1),
                )
            nrec = elup.tile([H, TB], F32, name="nrec", tag="nrec")
            nc.vector.tensor_add(
                nrec[:], nps[:], norm_loc[:, n * TB : (n + 1) * TB]
            )
            nc.vector.tensor_scalar_add(nrec[:], nrec[:], 1e-8)
            nc.vector.reciprocal_approx_fast(nrec[:], nrec[:])
            rb = constp.tile([H, TB], BF16, name=f"rb{n}")
            nc.vector.tensor_copy(rb[:], nrec[:])
            rbs.append(rb)

        # attention + projection, streamed per token chunk. attn is double-
        # buffered AND chunk n+1's attention matmuls are emitted before chunk
        # n's projection so the PE FIFO never waits on the DVE mul chain.
        def emit_attn(n):
            # reuses the (dead by now) xbt stream slots -- same shape
            attn_n = xbp.tile([128, KC, TB], BF16, name="attn_n", tag="xb")
            for j in range(KC):
                pps = mmps.tile([128, TB], F32, name="pps", tag="mm")
                nc.tensor.matmul(
                    pps[:],
                    lhsT=bd_p[:, j, :],
                    rhs=q_sb[:, j, n * TB : (n + 1) * TB],
                    start=True,
                    stop=True,
                )
                bps = mmps.tile([128, TB], F32, name="bps", tag="mm")
                nc.tensor.matmul(
                    bps[:],
                    lhsT=e_sb[:, j * 128 : (j + 1) * 128],
                    rhs=rbs[n][:],
                    start=True,
                    stop=True,
                )
                # DVE can read only one PSUM operand per op: stage the
                # broadcast through SBUF on the scalar engine first.
                bc_sb = elup.tile([128, TB], BF16, name="bc_sb", tag="bc_sb")
                nc.scalar.copy(bc_sb[:], bps[:])
                # total = peer (PSUM) + local (SBUF), then * bcast(recip)
                t2 = elup.tile([128, TB], BF16, name="t2", tag="t2")
                nc.vector.tensor_add(t2[:], pps[:], attn_loc[:, n, j, :])
                nc.vector.tensor_mul(attn_n[:, j, :], t2[:], bc_sb[:])
            return attn_n

        attn_tiles = {0: emit_attn(0)}
        for n in range(NT):
            if n + 1 < NT:
                attn_tiles[n + 1] = emit_attn(n + 1)
            attn_n = attn_tiles.pop(n)
            # output projection for this chunk: y[o, tok] = Wp^T @ attn + bp,
            # staged into half-row tiles for 4KB-line output DMAs
            for mh in range(2):
                o_h = attp.tile([128, 4 * TB], BF16, name="o_h", tag="o_h")
                for mi in range(4):
                    m = mh * 4 + mi
                    ops_ = mmps.tile([128, TB], F32, name="ops", tag="mm")
                    for j in range(KC):
                        nc.tensor.matmul(
                            ops_[:],
                            lhsT=wp_sb[:, j, m * 128 : (m + 1) * 128],
                            rhs=attn_n[:, j, :],
                            start=(j == 0),
                            stop=(j == KC - 1),
                        )
                    nc.scalar.activation(
                        o_h[:, mi * TB : (mi + 1) * TB],
                        ops_[:],
                        AF.Identity,
                        bias=bp_sb[:, m : m + 1],
                        scale=1.0,
                    )
                c0 = (n * KC + mh * 4) * TB
                eng = nc.scalar if mh == 0 else nc.sync
                eng.dma_start(y5[:, c0 : c0 + 4 * TB], o_h[:])


_NC_CACHE = {}


def get_nc():
    if "nc" not in _NC_CACHE:
        _NC_CACHE["nc"] = build_nc()
    return _NC_CACHE["nc"]


def _chunk_pack(w, dtype):
    """[C, C] weight -> [128, KC*C]: chunk k holds input-rows k*128..(k+1)*128."""
    return np.ascontiguousarray(
        w.reshape(KC, 128, C).transpose(1, 0, 2).reshape(128, KC * C)
    ).astype(dtype)


def make_in_maps(x, Wqkv, Wg, bg, Wp, bp):
    bf = ml_dtypes.bfloat16
    f8 = ml_dtypes.float8_e4m3fn
    x = np.asarray(x, dtype=np.float32)
    Wqkv = np.asarray(Wqkv, dtype=np.float32)
    Wg = np.asarray(Wg, dtype=np.float32)
    bg = np.asarray(bg, dtype=np.float32)
    Wp = np.asarray(Wp, dtype=np.float32)
    bp = np.asarray(bp, dtype=np.float32)

    def q8(w):
        return np.clip(w * WS, -240.0, 240.0).astype(f8)

    wq8 = _chunk_pack(q8(Wqkv[:, :C]), f8)
    wk8 = _chunk_pack(q8(Wqkv[:, C : 2 * C]), f8)
    wg8 = _chunk_pack(q8(Wg), f8)
    wv = _chunk_pack(Wqkv[:, 2 * C :], bf)
    wp = _chunk_pack(Wp, bf)
    bg2 = np.ascontiguousarray(bg.reshape(KC, 128).T)
    bp2 = np.ascontiguousarray(bp.reshape(KC, 128).T)
    e_allv = np.zeros((H, C), dtype=bf)
    for h in range(H):
        e_allv[h, h * D : (h + 1) * D] = 1.0

    xf = x.reshape(NCORES, T, C)
    in_maps = []
    for c in range(NCORES):
        xc = xf[c]                               # [T, C] fp32
        # xt8[n*128+p, k*TB+t] = x[n*TB+t, k*128+p]  (token-sliced blocks)
        xt8 = np.ascontiguousarray(
            np.clip(xc, -240.0, 240.0)
            .reshape(NT, TB, KC, 128).transpose(0, 3, 2, 1)
            .reshape(NT * 128, KC * TB)
        ).astype(f8)
        # xbt[g*128+p, k*TB+t] = x[g*TB+t, k*128+p]  (bf16 v-path stream tiles)
        xbt = np.ascontiguousarray(
            xc.reshape(NT, TB, KC, 128).transpose(0, 3, 2, 1)
            .reshape(NT * 128, KC * TB)
        ).astype(bf)
        in_maps.append(
            dict(
                xt8=xt8, xbt=xbt, wq8=wq8, wk8=wk8, wg8=wg8, wv=wv, wp=wp,
                bg2=bg2, bp2=bp2, e_all=e_allv,
            )
        )
    return in_maps


def unpack_y(y5):
    """y5 [128, NT*KC*TB] -> [T, C] fp32 (tokens, features)."""
    return (
        np.asarray(y5)
        .astype(np.float32)
        .reshape(128, NT, KC, TB)
        .transpose(1, 3, 2, 0)
        .reshape(T, C)
    )


def kernel(x, Wqkv, Wg, bg, Wp, bp, _collect_perf=None):
    nc = get_nc()
    in_maps = make_in_maps(x, Wqkv, Wg, bg, Wp, bp)
    kwargs = {}
    if _collect_perf is not None:
        kwargs = dict(trace=True)
        if _collect_perf.get("tmpdir"):
            kwargs["tmpdir"] = _collect_perf["tmpdir"]
    res = run_bass_kernel_spmd(
        nc, in_maps, core_ids=list(range(NCORES)), **kwargs
    )
    if _collect_perf is not None:
        _collect_perf["exec_time_ns"] = res.exec_time_ns
        _collect_perf["results"] = res
    out = np.empty((NCORES, T, C), dtype=np.float32)
    for c in range(NCORES):
        out[c] = unpack_y(res.results[c]["y5"])
    return out.reshape(B, N, C)
